# revision 8
# baseline (speedup 1.0000x reference)
"""Trainium2 Bass kernel for nn_DisOrFuncf_34067680591904.

Mathematical note: the reference computes
    out = inner + stop_gradient(fout - inner)
whose *value* is exactly fout (the `inner`/GOGradX machinery only shapes
gradients; fp32 check: max rel diff 1.2e-7, inside the reference's own
fp32-vs-fp64 envelope).  fout is a 3-layer MLP (784 -> 512 -> 256 -> 1,
leaky-relu 0.2, sigmoid) applied to x[:, 0, :].  The eval path
(is_train_g == 0) applies the same MLP to every (batch, level) row of x.

Strategy: pure data parallelism — shard MLP rows across the 8 cores
(32 rows/core train, 128 rows/core eval); weights replicated.

Per-core dataflow (R rows, fp32 end-to-end):
  L1  psum[R,512] = sum_c xT_c.T @ W1T_c   (stationary = xT chunk [128,R],
      moving = W1T chunk [128,512]; bias b1 rides a ones-row in the K=17
      tail chunk) -> leaky-relu on DVE -> d1 [R,512]
  PE transposes d1 -> d1T chunks [128,R]
  L2  psum[R,256] = sum_c2 d1T_c2.T @ W2T_c2 (+ ones-row x b2-row)
      -> leaky-relu -> d2 [R,256]
  L3  one DVE tensor_tensor_reduce: d3 = sum_o d2*w3 + b3 -> sigmoid (ACT)
A bf16 dummy-matmul burst warms the PE HAM clock gate while DMAs stream.
"""

import numpy as np

N_CORES = 8
BATCH, NC_LVL, D_IN, D_H1, D_H2 = 256, 4, 784, 512, 256
import os as _os
N_WARM = int(_os.environ.get("KERNEL_N_WARM", "8"))
_NO_TTR = bool(int(_os.environ.get("KERNEL_NO_TTR", "0")))
_NO_TRANS = bool(int(_os.environ.get("KERNEL_NO_TRANS", "0")))

_compiled = {}  # rows_per_core -> nc


def _build_nc(R: int):
    import concourse.bacc as bacc
    import concourse.tile as tile
    from concourse import mybir

    f32 = mybir.dt.float32
    bf16 = mybir.dt.bfloat16
    nc = bacc.Bacc("TRN2", target_bir_lowering=False, debug=False,
                   num_devices=N_CORES)

    # comb columns: [0:R]=identity_R, [R:R+256]=w3 bcast, [R+256]=b3 bcast,
    # row 0 of [R+257 : R+513] = b2
    CW = R + 513
    xt_d = nc.dram_tensor("xt", [128, 7 * R], f32, kind="ExternalInput")
    w1m_d = nc.dram_tensor("w1m", [3, 128, 1024], f32, kind="ExternalInput")
    w1t_d = nc.dram_tensor("w1t", [17, 512], f32, kind="ExternalInput")
    w2_d = nc.dram_tensor("w2", [128, 1024], f32, kind="ExternalInput")
    comb_d = nc.dram_tensor("comb", [R, CW], f32, kind="ExternalInput")
    out_d = nc.dram_tensor("out", [R, 1], f32, kind="ExternalOutput")

    with tile.TileContext(nc) as tc:
        with (
            tc.tile_pool(name="const", bufs=1) as cpool,
            tc.tile_pool(name="work", bufs=2) as wpool,
            tc.tile_pool(name="psum", bufs=1, space="PSUM") as ppool,
        ):
            # ---- PE warm-up: bf16 dummy matmuls on memset tiles ----
            if N_WARM:
                wa = cpool.tile([128, 128], bf16, tag="warm_a")
                nc.vector.memset(wa[:], 0.0)
                wb = cpool.tile([128, 512], bf16, tag="warm_b")
                nc.gpsimd.memset(wb[:], 0.0)
                psw = ppool.tile([128, 512], f32, tag="psw")
                for i in range(N_WARM):
                    nc.tensor.matmul(psw[:], wa[:], wb[:],
                                     start=(i == 0), stop=(i == N_WARM - 1))
                wsb = cpool.tile([1, 1], f32, tag="wsb")
                nc.vector.tensor_copy(wsb[:], psw[0:1, 0:1])

            # ---- DMAs (Sync: x + W1 main; Scalar: comb + W1 tail + W2) ----
            xt = cpool.tile([128, 7 * R], f32, tag="xt")
            nc.sync.dma_start(out=xt[:], in_=xt_d[:])
            w1 = []
            for i in range(3):
                t = cpool.tile([128, 1024], f32, tag=f"w1_{i}")
                nc.sync.dma_start(out=t[:], in_=w1m_d[i])
                w1.append(t)
            comb = cpool.tile([R, CW], f32, tag="comb")
            nc.scalar.dma_start(out=comb[:], in_=comb_d[:])
            w1t = cpool.tile([17, 512], f32, tag="w1t")
            nc.scalar.dma_start(out=w1t[:], in_=w1t_d[:])
            w2 = cpool.tile([128, 1024], f32, tag="w2")
            nc.scalar.dma_start(out=w2[:], in_=w2_d[:])

            ident = comb[:, 0:R]
            w3b = comb[:, R:R + 256]
            b3b = comb[:, R + 256:R + 257]
            b2row = comb[0:1, R + 257:R + 513]
            ones = cpool.tile([1, R], f32, tag="ones")
            nc.vector.memset(ones[:], 1.0)

            # ---- L1: d1 = lrelu(x @ W1T + b1)  [R, 512] ----
            ps1 = ppool.tile([R, 512], f32, tag="ps1")
            for c in range(6):
                nc.tensor.matmul(ps1[:], xt[:, R * c:R * c + R],
                                 w1[c // 2][:, 512 * (c % 2):512 * (c % 2) + 512],
                                 start=(c == 0), stop=False)
            nc.tensor.matmul(ps1[:], xt[0:17, 6 * R:7 * R], w1t[:],
                             start=False, stop=True)
            t1 = wpool.tile([R, 512], f32, tag="t1")
            nc.vector.tensor_scalar_mul(t1[:], ps1[:], 0.2)
            d1 = cpool.tile([R, 512], f32, tag="d1")
            nc.vector.tensor_max(d1[:], ps1[:], t1[:])

            # ---- transpose d1 -> d1T chunks [128, R] ----
            d1T = []
            for c2 in range(4):
                pst = ppool.tile([128, R], f32, tag="pst", bufs=2)
                nc.tensor.transpose(pst[:], d1[:, 128 * c2:128 * c2 + 128],
                                    ident)
                dt_ = cpool.tile([128, R], f32, tag=f"d1T_{c2}")
                nc.vector.tensor_copy(dt_[:], pst[:])
                d1T.append(dt_)

            # ---- L2: d2 = lrelu(d1 @ W2T + b2)  [R, 256] ----
            ps2 = ppool.tile([R, 256], f32, tag="ps2")
            for c2 in range(4):
                nc.tensor.matmul(ps2[:], d1T[c2][:],
                                 w2[:, 256 * c2:256 * c2 + 256],
                                 start=(c2 == 0), stop=False)
            nc.tensor.matmul(ps2[:], ones[:], b2row, start=False, stop=True)
            t2 = wpool.tile([R, 256], f32, tag="t2")
            nc.vector.tensor_scalar_mul(t2[:], ps2[:], 0.2)
            d2 = cpool.tile([R, 256], f32, tag="d2")
            nc.vector.tensor_max(d2[:], ps2[:], t2[:])

            # ---- L3: d3 = d2 . w3 + b3 ; sigmoid ----
            tr = wpool.tile([R, 256], f32, tag="tr")
            d3 = cpool.tile([R, 1], f32, tag="d3")
            nc.vector.tensor_mul(tr[:], d2[:], w3b)
            nc.vector.tensor_reduce(d3[:], tr[:], axis=mybir.AxisListType.X,
                                    op=mybir.AluOpType.add)
            ob = cpool.tile([R, 1], f32, tag="ob")
            nc.scalar.activation(ob[:], d3[:],
                                 mybir.ActivationFunctionType.Sigmoid,
                                 bias=b3b)
            nc.sync.dma_start(out=out_d[:], in_=ob[:])

    nc.compile()
    return nc


def _get_nc(R: int):
    if R not in _compiled:
        _compiled[R] = _build_nc(R)
    return _compiled[R]


def _pack_weights(W1, b1, W2, b2, W3, b3, R):
    f = np.float32
    # w1m[i][p, j*512+o] = W1T[128*(2i+j)+p, o] = W1[o, 128*(2i+j)+p]
    w1m = np.ascontiguousarray(
        W1[:, :768].reshape(512, 6, 128).transpose(1, 2, 0)   # [c, p, o]
        .reshape(3, 2, 128, 512).transpose(0, 2, 1, 3)        # [i, p, j, o]
        .reshape(3, 128, 1024), dtype=f)
    w1t = np.empty((17, 512), dtype=f)
    w1t[:16] = W1[:, 768:784].T
    w1t[16] = b1
    # w2[p, c2*256+o2] = W2T[128c2+p, o2] = W2[o2, 128c2+p]
    w2 = np.ascontiguousarray(
        W2.T.reshape(4, 128, 256).transpose(1, 0, 2).reshape(128, 1024),
        dtype=f)
    comb = np.zeros((R, R + 513), dtype=f)
    comb[:, :R] = np.eye(R, dtype=f)
    comb[:, R:R + 256] = W3[0][None, :]
    comb[:, R + 256] = b3[0]
    comb[0, R + 257:R + 513] = b2
    return w1m, w1t, w2, comb


def _pack_x(rows_c: np.ndarray, R: int):
    # xt[p, c*R+b] = rows_c[b, 128c+p] for c<6; tail chunk c=6: partitions
    # 0..15 = features 768..783, partition 16 = ones (bias row for L1)
    xt = np.zeros((128, 7 * R), dtype=np.float32)
    xt[:, :6 * R] = rows_c[:, :768].reshape(R, 6, 128).transpose(2, 1, 0) \
        .reshape(128, 6 * R)
    xt[:16, 6 * R:] = rows_c[:, 768:784].T
    xt[16, 6 * R:] = 1.0
    return xt


_trace_opts = None   # test harness hook: kwargs for run_bass_kernel_spmd
_last_results = None


def _run(rows: np.ndarray, R: int, weights) -> np.ndarray:
    global _last_results
    from concourse.bass_utils import run_bass_kernel_spmd

    nc = _get_nc(R)
    w1m, w1t, w2, comb = weights
    in_maps = []
    for c in range(N_CORES):
        xt = _pack_x(rows[c * R:(c + 1) * R], R)
        in_maps.append({"xt": xt, "w1m": w1m, "w1t": w1t,
                        "w2": w2, "comb": comb})
    res = run_bass_kernel_spmd(nc, in_maps, list(range(N_CORES)),
                               **(_trace_opts or {}))
    _last_results = res
    return np.concatenate([r["out"].reshape(R) for r in res.results])


def kernel(x, is_train_g, W1, b1, W2, b2, W3, b3):
    x = np.asarray(x, dtype=np.float32)
    args = [np.asarray(W1, np.float32), np.asarray(b1, np.float32),
            np.asarray(W2, np.float32), np.asarray(b2, np.float32),
            np.asarray(W3, np.float32), np.asarray(b3, np.float32)]
    if int(is_train_g):
        R = BATCH // N_CORES
        rows = np.ascontiguousarray(x[:, 0, :])          # [256, 784]
        out = _run(rows, R, _pack_weights(*args, R))
        return out.reshape(BATCH, 1)
    else:
        R = BATCH * NC_LVL // N_CORES
        rows = np.ascontiguousarray(x.reshape(BATCH * NC_LVL, D_IN))
        out = _run(rows, R, _pack_weights(*args, R))
        return out.reshape(BATCH, NC_LVL, 1)


# revision 9
# speedup vs baseline: 1.0655x; 1.0655x over previous
"""Trainium2 Bass kernel for nn_DisOrFuncf_34067680591904.

Mathematical note: the reference computes
    out = inner + stop_gradient(fout - inner)
whose *value* is exactly fout (the `inner`/GOGradX machinery only shapes
gradients; fp32 check: max rel diff 1.2e-7, inside the reference's own
fp32-vs-fp64 envelope).  fout is a 3-layer MLP (784 -> 512 -> 256 -> 1,
leaky-relu 0.2, sigmoid) applied to x[:, 0, :].  The eval path
(is_train_g == 0) applies the same MLP to every (batch, level) row of x.

Strategy: pure data parallelism — shard MLP rows across the 8 cores
(32 rows/core train, 128 rows/core eval); weights replicated.

Precision: matmuls run as bf16 hi/lo split pairs with fp32 PSUM
accumulation:  A @ W ~= Ah@Wh + Al@Wh + Ah@Wl  (the dropped Al@Wl term is
~2^-16 relative).  End-to-end max rel err vs the fp32 reference is ~1e-6
(measured), at 4x the matmul throughput of fp32 (which is double-pumped
on the PE) and with fast weight loads.

Per-core dataflow (R rows):
  L1  psum[R,512] += xT_c(h/l).T @ W1T_c(h/l)   3 terms x 7 k-chunks
      (stationary = xT chunk [<=128,R] bf16, moving = W1T chunk [.,512]
      bf16; bias b1 rides a ones-row in the K=17 tail chunk, split h/l)
      -> leaky-relu on DVE (fp32) -> d1 [R,512]
  PE transposes d1 -> psum [128,R]; DVE writes d1T_h (bf16 cast) and
      d1T_l (residual) straight from PSUM
  L2  psum[R,256] += d1T(h/l).T @ W2T(h/l) + ones x b2row(h/l)
      -> leaky-relu -> d2 [R,256] fp32
  L3  DVE: d3 = sum_o d2*w3 (fp32 mul+reduce); sigmoid(+b3) on ACT
A bf16 dummy-matmul burst warms the PE HAM clock gate while DMAs stream.
"""

import os as _os

import numpy as np
import ml_dtypes

N_CORES = 8
BATCH, NC_LVL, D_IN, D_H1, D_H2 = 256, 4, 784, 512, 256
N_WARM = int(_os.environ.get("KERNEL_N_WARM", "8"))

_compiled = {}  # rows_per_core -> nc


def _build_nc(R: int):
    import concourse.bacc as bacc
    import concourse.tile as tile
    from concourse import mybir

    f32 = mybir.dt.float32
    bf16 = mybir.dt.bfloat16
    nc = bacc.Bacc("TRN2", target_bir_lowering=False, debug=False,
                   num_devices=N_CORES)

    # comb columns (fp32): [0:R]=identity_R, [R:R+256]=w3 bcast, [R+256]=b3
    CW = R + 257
    xt_d = nc.dram_tensor("xt", [128, 14 * R], bf16, kind="ExternalInput")
    w1m_d = nc.dram_tensor("w1m", [3, 128, 2048], bf16, kind="ExternalInput")
    w1t_d = nc.dram_tensor("w1t", [17, 1024], bf16, kind="ExternalInput")
    w2_d = nc.dram_tensor("w2", [128, 2048], bf16, kind="ExternalInput")
    smb_d = nc.dram_tensor("smb", [1, 512], bf16, kind="ExternalInput")
    comb_d = nc.dram_tensor("comb", [R, CW], f32, kind="ExternalInput")
    out_d = nc.dram_tensor("out", [R, 1], f32, kind="ExternalOutput")

    with tile.TileContext(nc) as tc:
        with (
            tc.tile_pool(name="const", bufs=1) as cpool,
            tc.tile_pool(name="work", bufs=2) as wpool,
            tc.tile_pool(name="psum", bufs=1, space="PSUM") as ppool,
        ):
            # ---- PE warm-up: bf16 dummy matmuls on memset tiles ----
            if N_WARM:
                wa = cpool.tile([128, 128], bf16, tag="warm_a")
                nc.vector.memset(wa[:], 0.0)
                wb = cpool.tile([128, 512], bf16, tag="warm_b")
                nc.gpsimd.memset(wb[:], 0.0)
                psw = ppool.tile([128, 512], f32, tag="psw")
                for i in range(N_WARM):
                    nc.tensor.matmul(psw[:], wa[:], wb[:],
                                     start=(i == 0), stop=(i == N_WARM - 1))
                wsb = cpool.tile([1, 1], f32, tag="wsb")
                nc.vector.tensor_copy(wsb[:], psw[0:1, 0:1])

            # ---- DMAs (Sync: x + W1 main; Scalar: the rest) ----
            xt = cpool.tile([128, 14 * R], bf16, tag="xt")
            nc.sync.dma_start(out=xt[:], in_=xt_d[:])
            w1 = []
            for i in range(3):
                t = cpool.tile([128, 2048], bf16, tag=f"w1_{i}")
                nc.sync.dma_start(out=t[:], in_=w1m_d[i])
                w1.append(t)
            comb = cpool.tile([R, CW], f32, tag="comb")
            nc.scalar.dma_start(out=comb[:], in_=comb_d[:])
            w1t = cpool.tile([17, 1024], bf16, tag="w1t")
            nc.scalar.dma_start(out=w1t[:], in_=w1t_d[:])
            w2 = cpool.tile([128, 2048], bf16, tag="w2")
            nc.scalar.dma_start(out=w2[:], in_=w2_d[:])
            smb = cpool.tile([1, 512], bf16, tag="smb")
            nc.scalar.dma_start(out=smb[:], in_=smb_d[:])

            ident = comb[:, 0:R]
            w3b = comb[:, R:R + 256]
            b3b = comb[:, R + 256:R + 257]
            ones = cpool.tile([1, R], bf16, tag="ones")
            nc.vector.memset(ones[:], 1.0)

            def xh(c):
                return xt[:, R * c:R * c + R]

            def xl(c):
                return xt[:, 7 * R + R * c:7 * R + R * c + R]

            # ---- L1: d1 = lrelu(x @ W1T + b1)  [R, 512] ----
            ps1 = ppool.tile([R, 512], f32, tag="ps1")
            first = True
            for c in range(6):
                wh = w1[c // 2][:, 1024 * (c % 2):1024 * (c % 2) + 512]
                wl = w1[c // 2][:, 1024 * (c % 2) + 512:1024 * (c % 2) + 1024]
                nc.tensor.matmul(ps1[:], xh(c), wh, start=first, stop=False)
                first = False
                nc.tensor.matmul(ps1[:], xh(c), wl, start=False, stop=False)
                nc.tensor.matmul(ps1[:], xl(c), wh, start=False, stop=False)
            xth = xt[0:17, 6 * R:7 * R]
            xtl = xt[0:17, 13 * R:14 * R]
            nc.tensor.matmul(ps1[:], xth, w1t[:, 0:512],
                             start=False, stop=False)
            nc.tensor.matmul(ps1[:], xth, w1t[:, 512:1024],
                             start=False, stop=False)
            nc.tensor.matmul(ps1[:], xtl, w1t[:, 0:512],
                             start=False, stop=True)
            t1 = wpool.tile([R, 512], f32, tag="t1")
            nc.vector.tensor_scalar_mul(t1[:], ps1[:], 0.2)
            d1 = cpool.tile([R, 512], f32, tag="d1")
            nc.vector.tensor_max(d1[:], ps1[:], t1[:])

            # ---- transpose d1 -> d1T chunks, split to bf16 h/l ----
            d1h, d1l = [], []
            for c2 in range(4):
                pst = ppool.tile([128, R], f32, tag="pst", bufs=2)
                nc.tensor.transpose(pst[:], d1[:, 128 * c2:128 * c2 + 128],
                                    ident)
                th = cpool.tile([128, R], bf16, tag=f"d1h_{c2}")
                nc.vector.tensor_copy(th[:], pst[:])
                tl = cpool.tile([128, R], bf16, tag=f"d1l_{c2}")
                nc.vector.tensor_sub(tl[:], pst[:], th[:])
                d1h.append(th)
                d1l.append(tl)

            # ---- L2: d2 = lrelu(d1 @ W2T + b2)  [R, 256] ----
            ps2 = ppool.tile([R, 256], f32, tag="ps2")
            first = True
            for c2 in range(4):
                wh = w2[:, 512 * c2:512 * c2 + 256]
                wl = w2[:, 512 * c2 + 256:512 * c2 + 512]
                nc.tensor.matmul(ps2[:], d1h[c2][:], wh,
                                 start=first, stop=False)
                first = False
                nc.tensor.matmul(ps2[:], d1h[c2][:], wl,
                                 start=False, stop=False)
                nc.tensor.matmul(ps2[:], d1l[c2][:], wh,
                                 start=False, stop=False)
            nc.tensor.matmul(ps2[:], ones[:], smb[0:1, 0:256],
                             start=False, stop=False)
            nc.tensor.matmul(ps2[:], ones[:], smb[0:1, 256:512],
                             start=False, stop=True)
            t2 = wpool.tile([R, 256], f32, tag="t2")
            nc.vector.tensor_scalar_mul(t2[:], ps2[:], 0.2)
            d2 = cpool.tile([R, 256], f32, tag="d2")
            nc.vector.tensor_max(d2[:], ps2[:], t2[:])

            # ---- L3: d3 = d2 . w3 + b3 ; sigmoid ----
            tr = wpool.tile([R, 256], f32, tag="tr")
            d3 = cpool.tile([R, 1], f32, tag="d3")
            nc.vector.tensor_mul(tr[:], d2[:], w3b)
            nc.vector.tensor_reduce(d3[:], tr[:], axis=mybir.AxisListType.X,
                                    op=mybir.AluOpType.add)
            ob = cpool.tile([R, 1], f32, tag="ob")
            nc.scalar.activation(ob[:], d3[:],
                                 mybir.ActivationFunctionType.Sigmoid,
                                 bias=b3b)
            nc.sync.dma_start(out=out_d[:], in_=ob[:])

    nc.compile()
    return nc


def _get_nc(R: int):
    if R not in _compiled:
        _compiled[R] = _build_nc(R)
    return _compiled[R]


def _bf_split(a):
    h = a.astype(ml_dtypes.bfloat16)
    l = (a - h.astype(np.float32)).astype(ml_dtypes.bfloat16)
    return h, l


def _pack_weights(W1, b1, W2, b2, W3, b3, R):
    f = np.float32
    bf = ml_dtypes.bfloat16
    # W1T chunk layout [c, p, o]; then pack hi|lo per chunk, paired per DMA
    w1co = np.ascontiguousarray(
        W1[:, :768].reshape(512, 6, 128).transpose(1, 2, 0))  # [6,128,512]
    w1h, w1l = _bf_split(w1co)
    w1m = np.empty((3, 128, 2048), dtype=bf)
    for c in range(6):
        i, j = divmod(c, 2)
        w1m[i, :, 1024 * j:1024 * j + 512] = w1h[c]
        w1m[i, :, 1024 * j + 512:1024 * j + 1024] = w1l[c]
    # tail [17, 512]: 16 features + bias row
    w1tf = np.empty((17, 512), dtype=f)
    w1tf[:16] = W1[:, 768:784].T
    w1tf[16] = b1
    th, tl = _bf_split(w1tf)
    w1t = np.empty((17, 1024), dtype=bf)
    w1t[:, :512] = th
    w1t[:, 512:] = tl
    # W2T chunks [c2, p, o2] -> [p, c2*(h|l)]
    w2co = np.ascontiguousarray(W2.T.reshape(4, 128, 256))
    w2h, w2l = _bf_split(w2co)
    w2 = np.empty((128, 2048), dtype=bf)
    for c2 in range(4):
        w2[:, 512 * c2:512 * c2 + 256] = w2h[c2]
        w2[:, 512 * c2 + 256:512 * c2 + 512] = w2l[c2]
    # b2 row hi|lo
    bh, bl = _bf_split(b2.astype(f))
    smb = np.empty((1, 512), dtype=bf)
    smb[0, :256] = bh
    smb[0, 256:] = bl
    comb = np.zeros((R, R + 257), dtype=f)
    comb[:, :R] = np.eye(R, dtype=f)
    comb[:, R:R + 256] = W3[0][None, :]
    comb[:, R + 256] = b3[0]
    return w1m, w1t, w2, smb, comb


def _pack_x(rows_c: np.ndarray, R: int):
    # xt[p, c*R+b] (hi) / [p, 7R + c*R+b] (lo); tail chunk c=6 has the
    # ones bias row at partition 16 (hi=1, lo=0)
    xf = np.zeros((128, 7 * R), dtype=np.float32)
    xf[:, :6 * R] = rows_c[:, :768].reshape(R, 6, 128).transpose(2, 1, 0) \
        .reshape(128, 6 * R)
    xf[:16, 6 * R:] = rows_c[:, 768:784].T
    xf[16, 6 * R:] = 1.0
    h, l = _bf_split(xf)
    xt = np.empty((128, 14 * R), dtype=ml_dtypes.bfloat16)
    xt[:, :7 * R] = h
    xt[:, 7 * R:] = l
    return xt


_trace_opts = None   # test harness hook: kwargs for run_bass_kernel_spmd
_last_results = None


def _run(rows: np.ndarray, R: int, weights) -> np.ndarray:
    global _last_results
    from concourse.bass_utils import run_bass_kernel_spmd

    nc = _get_nc(R)
    w1m, w1t, w2, smb, comb = weights
    in_maps = []
    for c in range(N_CORES):
        xt = _pack_x(rows[c * R:(c + 1) * R], R)
        in_maps.append({"xt": xt, "w1m": w1m, "w1t": w1t,
                        "w2": w2, "smb": smb, "comb": comb})
    res = run_bass_kernel_spmd(nc, in_maps, list(range(N_CORES)),
                               **(_trace_opts or {}))
    _last_results = res
    return np.concatenate([r["out"].reshape(R) for r in res.results])


def kernel(x, is_train_g, W1, b1, W2, b2, W3, b3):
    x = np.asarray(x, dtype=np.float32)
    args = [np.asarray(W1, np.float32), np.asarray(b1, np.float32),
            np.asarray(W2, np.float32), np.asarray(b2, np.float32),
            np.asarray(W3, np.float32), np.asarray(b3, np.float32)]
    if int(is_train_g):
        R = BATCH // N_CORES
        rows = np.ascontiguousarray(x[:, 0, :])          # [256, 784]
        out = _run(rows, R, _pack_weights(*args, R))
        return out.reshape(BATCH, 1)
    else:
        R = BATCH * NC_LVL // N_CORES
        rows = np.ascontiguousarray(x.reshape(BATCH * NC_LVL, D_IN))
        out = _run(rows, R, _pack_weights(*args, R))
        return out.reshape(BATCH, NC_LVL, 1)


# revision 13
# speedup vs baseline: 1.2031x; 1.1292x over previous
"""Trainium2 Bass kernel for nn_DisOrFuncf_34067680591904.

Mathematical note: the reference computes
    out = inner + stop_gradient(fout - inner)
whose *value* is exactly fout (the `inner`/GOGradX machinery only shapes
gradients; fp32 check: max rel diff 1.2e-7, inside the reference's own
fp32-vs-fp64 envelope).  fout is a 3-layer MLP (784 -> 512 -> 256 -> 1,
leaky-relu 0.2, sigmoid) applied to x[:, 0, :].  The eval path
(is_train_g == 0) applies the same MLP to every (batch, level) row of x.

Strategy: pure data parallelism — shard MLP rows across the 8 cores
(32 rows/core train, 128 rows/core eval); weights replicated.

Precision: matmuls run as bf16 hi/lo split pairs with fp32 PSUM
accumulation:  A @ W ~= Ah@Wh + Al@Wh + Ah@Wl  (the dropped Al@Wl term is
~2^-16 relative).  End-to-end max rel err vs the fp32 reference is ~1e-6
(measured), at 4x the matmul throughput of fp32 (which is double-pumped
on the PE) and with fast weight loads.

Per-core dataflow (R rows):
  L1  psum[R,512] += xT_c(h/l).T @ W1T_c(h/l)   3 terms x 7 k-chunks
      (stationary = xT chunk [<=128,R] bf16, moving = W1T chunk [.,512]
      bf16; bias b1 rides a ones-row in the K=17 tail chunk, split h/l)
      -> leaky-relu on DVE (fp32) -> d1 [R,512]
  PE transposes d1 -> psum [128,R]; DVE writes d1T_h (bf16 cast) and
      d1T_l (residual) straight from PSUM
  L2  psum[R,256] += d1T(h/l).T @ W2T(h/l) + ones x b2row(h/l)
      -> leaky-relu -> d2 [R,256] fp32
  L3  DVE: d3 = sum_o d2*w3 (fp32 mul+reduce); sigmoid(+b3) on ACT
A bf16 dummy-matmul burst warms the PE HAM clock gate while DMAs stream.
"""

import os as _os

import numpy as np
import ml_dtypes

N_CORES = 8
BATCH, NC_LVL, D_IN, D_H1, D_H2 = 256, 4, 784, 512, 256
N_WARM = int(_os.environ.get("KERNEL_N_WARM", "4"))

_compiled = {}  # rows_per_core -> nc


def _build_nc(R: int):
    import concourse.bacc as bacc
    import concourse.tile as tile
    from concourse import mybir

    f32 = mybir.dt.float32
    bf16 = mybir.dt.bfloat16
    nc = bacc.Bacc("TRN2", target_bir_lowering=False, debug=False,
                   num_devices=N_CORES)

    # comb columns (fp32): [0:R]=identity_R, [R:R+256]=w3 bcast, [R+256]=b3
    CW = R + 257
    xt_d = nc.dram_tensor("xt", [128, 14 * R], bf16, kind="ExternalInput")
    w1m_d = nc.dram_tensor("w1m", [3, 128, 2048], bf16, kind="ExternalInput")
    w1t_d = nc.dram_tensor("w1t", [17, 1024], bf16, kind="ExternalInput")
    w2_d = nc.dram_tensor("w2", [128, 2048], bf16, kind="ExternalInput")
    smb_d = nc.dram_tensor("smb", [1, 512], bf16, kind="ExternalInput")
    comb_d = nc.dram_tensor("comb", [R, CW], f32, kind="ExternalInput")
    out_d = nc.dram_tensor("out", [R, 1], f32, kind="ExternalOutput")

    with tile.TileContext(nc) as tc:
        with (
            tc.tile_pool(name="const", bufs=1) as cpool,
            tc.tile_pool(name="work", bufs=2) as wpool,
            tc.tile_pool(name="psum", bufs=1, space="PSUM") as ppool,
        ):
            # ---- PE warm-up: bf16 dummy matmuls on memset tiles ----
            if N_WARM:
                wa = cpool.tile([128, 128], bf16, tag="warm_a")
                nc.vector.memset(wa[:], 0.0)
                wb = cpool.tile([128, 512], bf16, tag="warm_b")
                nc.gpsimd.memset(wb[:], 0.0)
                psw = ppool.tile([128, 512], f32, tag="psw")
                for i in range(N_WARM):
                    nc.tensor.matmul(psw[:], wa[:], wb[:],
                                     start=(i == 0), stop=(i == N_WARM - 1))
                wsb = cpool.tile([1, 1], f32, tag="wsb")
                nc.vector.tensor_copy(wsb[:], psw[0:1, 0:1])

            # ---- DMAs (Sync: W1 main; Scalar: x + the rest) ----
            w1 = []
            for i in range(3):
                t = cpool.tile([128, 2048], bf16, tag=f"w1_{i}")
                nc.sync.dma_start(out=t[:], in_=w1m_d[i])
                w1.append(t)
            xt = cpool.tile([128, 14 * R], bf16, tag="xt")
            nc.scalar.dma_start(out=xt[:], in_=xt_d[:])
            comb = cpool.tile([R, CW], f32, tag="comb")
            nc.scalar.dma_start(out=comb[:], in_=comb_d[:])
            w1t = cpool.tile([17, 1024], bf16, tag="w1t")
            nc.scalar.dma_start(out=w1t[:], in_=w1t_d[:])
            w2 = cpool.tile([128, 2048], bf16, tag="w2")
            nc.scalar.dma_start(out=w2[:], in_=w2_d[:])
            smb = cpool.tile([1, 512], bf16, tag="smb")
            nc.scalar.dma_start(out=smb[:], in_=smb_d[:])

            ident = comb[:, 0:R]
            w3b = comb[:, R:R + 256]
            b3b = comb[:, R + 256:R + 257]
            ones = cpool.tile([1, R], bf16, tag="ones")
            nc.vector.memset(ones[:], 1.0)

            def xh(c):
                return xt[:, R * c:R * c + R]

            def xl(c):
                return xt[:, 7 * R + R * c:7 * R + R * c + R]

            # ---- L1: d1 = lrelu(x @ W1T + b1)  [R, 512] ----
            ps1 = ppool.tile([R, 512], f32, tag="ps1")
            first = True
            for c in range(6):
                wh = w1[c // 2][:, 1024 * (c % 2):1024 * (c % 2) + 512]
                wl = w1[c // 2][:, 1024 * (c % 2) + 512:1024 * (c % 2) + 1024]
                nc.tensor.matmul(ps1[:], xh(c), wh, start=first, stop=False)
                first = False
                nc.tensor.matmul(ps1[:], xh(c), wl, start=False, stop=False)
                nc.tensor.matmul(ps1[:], xl(c), wh, start=False, stop=False)
            xth = xt[0:17, 6 * R:7 * R]
            xtl = xt[0:17, 13 * R:14 * R]
            nc.tensor.matmul(ps1[:], xth, w1t[:, 0:512],
                             start=False, stop=False)
            nc.tensor.matmul(ps1[:], xth, w1t[:, 512:1024],
                             start=False, stop=False)
            nc.tensor.matmul(ps1[:], xtl, w1t[:, 0:512],
                             start=False, stop=True)
            # ---- per 128-col chunk: lrelu -> transpose -> split h/l ----
            # (pipelines DVE/PE work chunk by chunk)
            d1h, d1l = [], []
            for c2 in range(4):
                sl = slice(128 * c2, 128 * c2 + 128)
                t1 = wpool.tile([R, 128], f32, tag="t1")
                nc.vector.tensor_scalar_mul(t1[:], ps1[:, sl], 0.2)
                d1c = wpool.tile([R, 128], f32, tag="d1c", bufs=3)
                nc.vector.tensor_max(d1c[:], ps1[:, sl], t1[:])
                pst = ppool.tile([128, R], f32, tag="pst", bufs=2)
                nc.tensor.transpose(pst[:], d1c[:], ident)
                th = cpool.tile([128, R], bf16, tag=f"d1h_{c2}")
                nc.vector.tensor_copy(th[:], pst[:])
                tl = cpool.tile([128, R], bf16, tag=f"d1l_{c2}")
                nc.vector.tensor_sub(tl[:], pst[:], th[:])
                d1h.append(th)
                d1l.append(tl)

            # ---- L2: d2 = lrelu(d1 @ W2T + b2)  [R, 256] ----
            ps2 = ppool.tile([R, 256], f32, tag="ps2")
            first = True
            for c2 in range(4):
                wh = w2[:, 512 * c2:512 * c2 + 256]
                wl = w2[:, 512 * c2 + 256:512 * c2 + 512]
                nc.tensor.matmul(ps2[:], d1h[c2][:], wh,
                                 start=first, stop=False)
                first = False
                nc.tensor.matmul(ps2[:], d1h[c2][:], wl,
                                 start=False, stop=False)
                nc.tensor.matmul(ps2[:], d1l[c2][:], wh,
                                 start=False, stop=False)
            nc.tensor.matmul(ps2[:], ones[:], smb[0:1, 0:256],
                             start=False, stop=False)
            nc.tensor.matmul(ps2[:], ones[:], smb[0:1, 256:512],
                             start=False, stop=True)
            t2 = wpool.tile([R, 256], f32, tag="t2")
            nc.vector.tensor_scalar_mul(t2[:], ps2[:], 0.2)
            d2 = cpool.tile([R, 256], f32, tag="d2")
            nc.vector.tensor_max(d2[:], ps2[:], t2[:])

            # ---- L3: d3 = d2 . w3 + b3 ; sigmoid ----
            tr = wpool.tile([R, 256], f32, tag="tr")
            d3 = cpool.tile([R, 1], f32, tag="d3")
            nc.vector.scalar_tensor_tensor(
                tr[:], d2[:], 1.0, w3b,
                op0=mybir.AluOpType.mult, op1=mybir.AluOpType.mult,
                accum_out=d3[:])
            ob = cpool.tile([R, 1], f32, tag="ob")
            nc.scalar.activation(ob[:], d3[:],
                                 mybir.ActivationFunctionType.Sigmoid,
                                 bias=b3b)
            nc.sync.dma_start(out=out_d[:], in_=ob[:])

    nc.compile()
    return nc


def _get_nc(R: int):
    if R not in _compiled:
        _compiled[R] = _build_nc(R)
    return _compiled[R]


def _bf_split(a):
    h = a.astype(ml_dtypes.bfloat16)
    l = (a - h.astype(np.float32)).astype(ml_dtypes.bfloat16)
    return h, l


def _pack_weights(W1, b1, W2, b2, W3, b3, R):
    f = np.float32
    bf = ml_dtypes.bfloat16
    # W1T chunk layout [c, p, o]; then pack hi|lo per chunk, paired per DMA
    w1co = np.ascontiguousarray(
        W1[:, :768].reshape(512, 6, 128).transpose(1, 2, 0))  # [6,128,512]
    w1h, w1l = _bf_split(w1co)
    w1m = np.empty((3, 128, 2048), dtype=bf)
    for c in range(6):
        i, j = divmod(c, 2)
        w1m[i, :, 1024 * j:1024 * j + 512] = w1h[c]
        w1m[i, :, 1024 * j + 512:1024 * j + 1024] = w1l[c]
    # tail [17, 512]: 16 features + bias row
    w1tf = np.empty((17, 512), dtype=f)
    w1tf[:16] = W1[:, 768:784].T
    w1tf[16] = b1
    th, tl = _bf_split(w1tf)
    w1t = np.empty((17, 1024), dtype=bf)
    w1t[:, :512] = th
    w1t[:, 512:] = tl
    # W2T chunks [c2, p, o2] -> [p, c2*(h|l)]
    w2co = np.ascontiguousarray(W2.T.reshape(4, 128, 256))
    w2h, w2l = _bf_split(w2co)
    w2 = np.empty((128, 2048), dtype=bf)
    for c2 in range(4):
        w2[:, 512 * c2:512 * c2 + 256] = w2h[c2]
        w2[:, 512 * c2 + 256:512 * c2 + 512] = w2l[c2]
    # b2 row hi|lo
    bh, bl = _bf_split(b2.astype(f))
    smb = np.empty((1, 512), dtype=bf)
    smb[0, :256] = bh
    smb[0, 256:] = bl
    comb = np.zeros((R, R + 257), dtype=f)
    comb[:, :R] = np.eye(R, dtype=f)
    comb[:, R:R + 256] = W3[0][None, :]
    comb[:, R + 256] = b3[0]
    return w1m, w1t, w2, smb, comb


def _pack_x(rows_c: np.ndarray, R: int):
    # xt[p, c*R+b] (hi) / [p, 7R + c*R+b] (lo); tail chunk c=6 has the
    # ones bias row at partition 16 (hi=1, lo=0)
    xf = np.zeros((128, 7 * R), dtype=np.float32)
    xf[:, :6 * R] = rows_c[:, :768].reshape(R, 6, 128).transpose(2, 1, 0) \
        .reshape(128, 6 * R)
    xf[:16, 6 * R:] = rows_c[:, 768:784].T
    xf[16, 6 * R:] = 1.0
    h, l = _bf_split(xf)
    xt = np.empty((128, 14 * R), dtype=ml_dtypes.bfloat16)
    xt[:, :7 * R] = h
    xt[:, 7 * R:] = l
    return xt


_trace_opts = None   # test harness hook: kwargs for run_bass_kernel_spmd
_last_results = None


def _run(rows: np.ndarray, R: int, weights) -> np.ndarray:
    global _last_results
    from concourse.bass_utils import run_bass_kernel_spmd

    nc = _get_nc(R)
    w1m, w1t, w2, smb, comb = weights
    in_maps = []
    for c in range(N_CORES):
        xt = _pack_x(rows[c * R:(c + 1) * R], R)
        in_maps.append({"xt": xt, "w1m": w1m, "w1t": w1t,
                        "w2": w2, "smb": smb, "comb": comb})
    res = run_bass_kernel_spmd(nc, in_maps, list(range(N_CORES)),
                               **(_trace_opts or {}))
    _last_results = res
    return np.concatenate([r["out"].reshape(R) for r in res.results])


def kernel(x, is_train_g, W1, b1, W2, b2, W3, b3):
    x = np.asarray(x, dtype=np.float32)
    args = [np.asarray(W1, np.float32), np.asarray(b1, np.float32),
            np.asarray(W2, np.float32), np.asarray(b2, np.float32),
            np.asarray(W3, np.float32), np.asarray(b3, np.float32)]
    if int(is_train_g):
        R = BATCH // N_CORES
        rows = np.ascontiguousarray(x[:, 0, :])          # [256, 784]
        out = _run(rows, R, _pack_weights(*args, R))
        return out.reshape(BATCH, 1)
    else:
        R = BATCH * NC_LVL // N_CORES
        rows = np.ascontiguousarray(x.reshape(BATCH * NC_LVL, D_IN))
        out = _run(rows, R, _pack_weights(*args, R))
        return out.reshape(BATCH, NC_LVL, 1)
